# revision 12
# baseline (speedup 1.0000x reference)
"""Trainium2 Bass kernel for nn_ContrastiveLoss_22333829940001.

Strategy (data-parallel over batch, 8 cores; core b owns batch b):
  Host prep builds, per core and per step k, a pre-gathered pair-ordered
  operand stream in TRN fp8 (e4m3): for each context position n (0..2047,
  padded past L=T-k with the zero row) there are 11 slots: j=0..9 the
  gathered negative z rows z_flat[neg_idx[k-1, bL+n, j]], j=10 the positive
  row z_flat[b*T + n + k].  The stream is stored transposed (channels on
  partitions): gt[(s,c), p, (g, j, n')] = z_flat[idx, g*128+p] for n-chunks
  of 512.  cpt[s, p, (g, n)] = predictions[s, b, g*128+p, n] (fp8).

  Device, per step: GPSIMD cast-DMAs gt chunks / cpt / zt fp8->bf16 into
  SBUF; DVE multiplies each chunk by the broadcast cpt columns (2x bf16
  mode); positives multiply the SBUF-resident shifted zt against cpt.
  TensorE reduces the 512 channels with ones[128,32]-stationary matmuls
  (explicit 32-aligned tile_position), 32 replicated PSUM rows per (j,
  chunk) group; ScalarE copies each PSUM bank to SBUF and SBUF->SBUF DMAs
  pack one row per group into a [68, 512] sims tile (rows 0..39 negatives,
  64..67 positives - engine APs need 32-aligned partition bases). ScalarE
  computes softplus(x) = relu(+-x) + ln1p(exp(-|x|)) with per-partition
  accumulation into a [68, 24] output; host combines in float64 with
  deterministic ln(2) pad corrections.
"""

import os
import sys

sys.path.insert(0, "/opt/trn_rl_repo")

import numpy as np
import ml_dtypes

import concourse.bass as bass
import concourse.tile as tile
from concourse import bacc, mybir
from concourse import bass_utils

N_CORES = 8
B, C, T = 8, 512, 2048
K_STEPS = 12
NUM_NEG = 10
NJ = NUM_NEG           # negative slots per position (positives from zt)
NCHUNK = 4             # n-chunks per step
NB = T // NCHUNK       # 512 positions per chunk
FREE = 4 * NJ * NB     # free elems per partition per chunk (g, j, n')
NROWS = 68             # sims rows: 0..39 negatives, 64..67 positives
TP = T + 16            # padded time length of the resident z tile
ZPAD = B * T           # zero-row index in the padded z table
LN2 = float(np.log(2.0))

_compiled = None


def _build_program():
    nc = bacc.Bacc("TRN2", target_bir_lowering=False, debug=False,
                   num_devices=N_CORES)
    AF = mybir.ActivationFunctionType
    bf16 = mybir.dt.bfloat16
    f8 = mybir.dt.float8e4
    f32 = mybir.dt.float32

    gt_d = nc.dram_tensor("gt", [K_STEPS * NCHUNK, 128, FREE], f8,
                          kind="ExternalInput").ap()
    cpt_d = nc.dram_tensor("cpt", [K_STEPS, 128, 4 * T], f8,
                           kind="ExternalInput").ap()
    zt_d = nc.dram_tensor("zt", [128, 4 * TP], f8,
                          kind="ExternalInput").ap()
    out_d = nc.dram_tensor("out", [NROWS, 2 * K_STEPS], f32,
                           kind="ExternalOutput").ap()

    with tile.TileContext(nc) as tc:
        with (
            tc.tile_pool(name="gtp", bufs=2) as gtp,
            tc.tile_pool(name="pp", bufs=2) as pp,
            tc.tile_pool(name="cptp", bufs=2) as cptp,
            tc.tile_pool(name="posp", bufs=1) as posp,
            tc.tile_pool(name="gt8p", bufs=1) as gt8p,
            tc.tile_pool(name="bip", bufs=1) as bip,
            tc.tile_pool(name="psp", bufs=8, space="PSUM") as psp,
            tc.tile_pool(name="scr", bufs=2) as scr,
            tc.tile_pool(name="sp", bufs=1) as sp,
            tc.tile_pool(name="outp", bufs=1) as outp,
        ):
            ones = outp.tile([128, 32], bf16, tag="ones")
            nc.gpsimd.memset(ones[:], 1.0)
            c80 = outp.tile([128, 1], f32, tag="c80")
            nc.gpsimd.memset(c80[:], 80.0)
            cm80 = outp.tile([128, 1], f32, tag="cm80")
            nc.gpsimd.memset(cm80[:], -80.0)
            out_sb = outp.tile([NROWS, 2 * K_STEPS], f32, tag="out")
            zt_sb = outp.tile([128, 4, TP], bf16, tag="zt")
            nc.gpsimd.dma_start(zt_sb[:], zt_d[:])

            for s in range(K_STEPS):
                cpt_sb = cptp.tile([128, 4, T], bf16, tag="cpt")
                nc.gpsimd.dma_start(cpt_sb[:], cpt_d[s])
                sims = scr.tile([NROWS, NB], f32, tag="sims")
                # positives: sim_pos[n] = sum_c zt[c, n+k] * cpt[c, n]
                p_pos = posp.tile([128, 4, T], bf16, tag="ppos")
                nc.vector.tensor_tensor(
                    p_pos[:], zt_sb[:, :, s + 1:s + 1 + T], cpt_sb[:],
                    mybir.AluOpType.mult)
                pt_pos = psp.tile([128, NB], f32, tag="ps")
                for c in range(NCHUNK):
                    for g in range(4):
                        nc.tensor.matmul(
                            pt_pos[32 * c:32 * c + 32, :],
                            ones[:, :32],
                            p_pos[:, g, c * NB:(c + 1) * NB],
                            start=(g == 0), stop=(g == 3),
                            tile_position=(0, 32 * c),
                        )
                bimg_pos = bip.tile([128, NB], f32, tag="bimg")
                nc.scalar.activation(bimg_pos[:], pt_pos[:], AF.Identity)
                nc.sync.dma_start(sims[64:68, :], bimg_pos[0:97:32, :])
                for c in range(NCHUNK):
                    gt_sb = gtp.tile([128, 4, NJ, NB], bf16, tag="gt")
                    cp_chunk = cpt_sb[:, :, c * NB:(c + 1) * NB]
                    if c < NCHUNK - 1:
                        nc.gpsimd.dma_start(gt_sb[:], gt_d[s * NCHUNK + c])
                    else:
                        # last chunk: channel group 3 rides plain-fp8 HWDGE
                        # (half the fabric bytes, off the SWDGE cast queue)
                        # and multiplies at 1x from fp8 directly.
                        nc.gpsimd.dma_start(
                            gt_sb[:, 0:3, :, :],
                            gt_d[s * NCHUNK + c][:, 0:3 * NJ * NB])
                        gt8 = gt8p.tile([128, 1, NJ, NB], f8, tag="gt8")
                        nc.sync.dma_start(
                            gt8[:], gt_d[s * NCHUNK + c][:, 3 * NJ * NB:])
                    p_tiles = []
                    for j0, njh in ((0, 5), (5, 5)):
                        p_sb = pp.tile([128, 4, 5, NB], bf16, tag="p")
                        p_tiles.append((j0, njh, p_sb))
                        if c < NCHUNK - 1:
                            in1 = cp_chunk.unsqueeze(2).broadcast_to(
                                (128, 4, njh, NB))
                            nc.vector.tensor_tensor(
                                p_sb[:, :, :njh, :],
                                gt_sb[:, :, j0:j0 + njh, :],
                                in1, mybir.AluOpType.mult)
                        else:
                            in1a = cp_chunk[:, 0:3, :].unsqueeze(2).broadcast_to(
                                (128, 3, njh, NB))
                            nc.vector.tensor_tensor(
                                p_sb[:, 0:3, :njh, :],
                                gt_sb[:, 0:3, j0:j0 + njh, :],
                                in1a, mybir.AluOpType.mult)
                            in1b = cp_chunk[:, 3:4, :].unsqueeze(2).broadcast_to(
                                (128, 1, njh, NB))
                            nc.vector.tensor_tensor(
                                p_sb[:, 3:4, :njh, :],
                                gt8[:, :, j0:j0 + njh, :],
                                in1b, mybir.AluOpType.mult)
                    # Column sums via ones-stationary matmuls: group j of this
                    # chunk -> 32 replicated PSUM rows at base 32*(j%4), 3
                    # bank tiles per chunk. ACT copies each bank to SBUF;
                    # SBUF->SBUF DMAs (no partition alignment rules) pack row
                    # 32*i of each bank into sims row j*4+c (pos j=10 ->
                    # 64+c so ACT slices stay 32-aligned).
                    for t in range(3):
                        jlist = [j for j in range(4 * t, min(4 * t + 4, NJ))]
                        pt = psp.tile([128, NB], f32, tag="ps")
                        for j in jlist:
                            base = 32 * (j % 4)
                            j0, njh, p_sb = (
                                p_tiles[0] if j < 5 else p_tiles[1])
                            jj = j - j0
                            for g in range(4):
                                nc.tensor.matmul(
                                    pt[base:base + 32, :],
                                    ones[:, :32],
                                    p_sb[:, g, jj, :],
                                    start=(g == 0), stop=(g == 3),
                                    tile_position=(0, base),
                                )
                        bimg = bip.tile([128, NB], f32, tag="bimg")
                        nc.scalar.activation(bimg[:], pt[:], AF.Identity)
                        ng = len(jlist)
                        r0 = 16 * t + c
                        nc.sync.dma_start(
                            sims[r0:r0 + 4 * (ng - 1) + 1:4, :],
                            bimg[0:32 * (ng - 1) + 1:32, :])
                # softplus(x) = relu(s*x) + ln(1 + exp(-min(|x|, 80)))
                a = sp.tile([NROWS, NB], f32, tag="c0")
                nc.scalar.activation(a[:], sims[:], AF.Abs)
                r1 = sp.tile([NROWS, NB], f32, tag="c1")
                nc.scalar.activation(r1[:], a[:], AF.Relu, scale=-1.0,
                                     bias=c80[0:NROWS])
                t_ = sp.tile([NROWS, NB], f32, tag="c0")
                nc.scalar.activation(t_[:], r1[:], AF.Exp, bias=cm80[0:NROWS])
                u = sp.tile([NROWS, NB], f32, tag="c1")
                nc.scalar.activation(u[:], t_[:], AF.Ln, bias=1.0,
                                     accum_out=out_sb[:, 2 * s:2 * s + 1])
                rn = sp.tile([NROWS, NB], f32, tag="c0")
                nc.scalar.activation(rn[0:40, :], sims[0:40, :], AF.Relu,
                                     accum_out=out_sb[0:40, 2 * s + 1:2 * s + 2])
                nc.scalar.activation(rn[64:NROWS, :], sims[64:NROWS, :],
                                     AF.Relu, scale=-1.0,
                                     accum_out=out_sb[64:NROWS,
                                                      2 * s + 1:2 * s + 2])

            nc.sync.dma_start(out_d[:], out_sb[:])

    nc.compile()
    return nc


def _host_prep(z, c, predictions, neg_indices):
    """Build per-core input maps. `c` is unused by the reference."""
    del c
    f8 = ml_dtypes.float8_e4m3
    # zT8: [C, B*T + 1] fp8, channel-major, trailing zero column for pads.
    zt = np.transpose(np.asarray(z), (1, 0, 2)).reshape(C, B * T)
    zt8 = np.zeros((C, B * T + 1), dtype=f8)
    zt8[:, :B * T] = zt.astype(f8)
    pred8 = np.asarray(predictions).astype(f8)  # [K, B, C, T]
    neg = np.asarray(neg_indices)

    in_maps = []
    for b in range(N_CORES):
        gt = np.empty((K_STEPS * NCHUNK, 128, FREE), dtype=f8)
        for s in range(K_STEPS):
            k = s + 1
            L = T - k
            idxt = np.full((NJ, T), ZPAD, dtype=np.int64)
            idxt[:, :L] = neg[s, b * L:(b + 1) * L, :].T
            g = zt8[:, idxt]                       # [512, 10, 2048]
            g = g.reshape(4, 128, NJ, NCHUNK, NB)  # [g, p, j, c, n']
            g = np.ascontiguousarray(np.transpose(g, (3, 1, 0, 2, 4)))
            gt[s * NCHUNK:(s + 1) * NCHUNK] = g.reshape(NCHUNK, 128, FREE)
        cpt = np.ascontiguousarray(
            np.transpose(pred8[:, b].reshape(K_STEPS, 4, 128, T),
                         (0, 2, 1, 3))).reshape(K_STEPS, 128, 4 * T)
        zt = np.zeros((128, 4, TP), dtype=f8)
        zt[:, :, :T] = np.transpose(
            zt8[:, b * T:(b + 1) * T].reshape(4, 128, T), (1, 0, 2))
        in_maps.append({"gt": gt, "cpt": cpt, "zt": zt.reshape(128, 4 * TP)})
    return in_maps


def _combine(partials_per_core):
    """partials: per core [44, 24] f32 -> scalar loss (float64 host math).

    col 2s   = per-row sum of ln1p(exp(-|sim|))
    col 2s+1 = per-row sum of relu(sim) (rows 0..39, negatives)
               or relu(-sim) (rows 40..43, positives)
    Pad slots (n >= L) have sim == 0 and contribute exactly ln(2) each.
    """
    total = 0.0
    for s in range(K_STEPS):
        k = s + 1
        L = T - k
        neg_sum = 0.0
        pos_sum = 0.0
        for p in partials_per_core:
            p64 = p.astype(np.float64)
            neg_sum += p64[0:40, 2 * s].sum() + p64[0:40, 2 * s + 1].sum()
            pos_sum += p64[64:NROWS, 2 * s].sum() + p64[64:NROWS, 2 * s + 1].sum()
        neg_sum -= N_CORES * NUM_NEG * (T - L) * LN2
        pos_sum -= N_CORES * (T - L) * LN2
        total += neg_sum / (N_CORES * L * NUM_NEG) + pos_sum / (N_CORES * L)
    return np.float32(total / K_STEPS)


def run(inputs, trace=False):
    global _compiled
    if _compiled is None:
        _compiled = _build_program()
    nc = _compiled
    in_maps = _host_prep(**inputs)
    res = bass_utils.run_bass_kernel_spmd(
        nc, in_maps, core_ids=list(range(N_CORES)), trace=trace)
    loss = _combine([res.results[i]["out"] for i in range(N_CORES)])
    return loss, res


def kernel(**inputs) -> np.ndarray:
    inputs = {k: np.asarray(v) for k, v in inputs.items()}
    loss, _ = run(inputs, trace=bool(int(os.environ.get("KERNEL_TRACE", "0"))))
    return np.asarray(loss, dtype=np.float32)
